# revision 27
# baseline (speedup 1.0000x reference)
"""Trainium2 Bass kernel for the batched Kalman filter problem.

Problem: emissions [2048, 512, 4], m0 [2048, 8], P0 [2048, 8, 8] (identical
across batch in the reference setup), A/Q [8,8], H [4,8], R [4,4].
Outputs: marginal_log_likelihood [2048], filtered_means [2048, 512, 8],
filtered_covariances [2048, 512, 8, 8].

Strategy
--------
P0 is identical for every batch row, so the covariance/gain recursion
(Pp = A P A' + Q, S = H Pp H' + R, K, Pf) is batch-independent: the filtered
covariances are one shared [T, D, D] trajectory and the per-batch work is

    means:  m_t = G_t m_{t-1} + K_t y_t      (time-varying linear recurrence)
    loglik: ll(b) = -sum_t ||z_t||^2/2 - C,  z_t = L_t^{-1}(y_t - H A m_{t-1})

with G_t, K_t, L_t shared. The tiny sequential T-step recursion of 8x8
matrices runs on host in float64; everything O(B*T) runs on device.

Time is chunked into blocks of 16 steps. Within a block, the stacked means
[16*8, B] and stacked whitened innovations [16*4, B] are linear in
(m_blockstart, y_block), so each is exactly two TensorEngine matmuls with
host-precomputed transfer operators. The sequential dependency is only the
[8, B] carry between blocks. Log-likelihood accumulates in PSUM via a
ones-vector matmul over squared z. The shared covariance trajectory is
broadcast-written from SBUF to every batch row of the output (the memory-
bound bulk: ~32 MB per core).

Batch 2048 is sharded 8 ways (pure data parallel, 256 rows/core); each core
runs the identical program on its shard.
"""

import numpy as np

B, T, D, E = 2048, 512, 8, 4
NCORES = 8
BC = B // NCORES  # 256 batch rows per core
LBLK = 16
NBLK = T // LBLK  # 32
LN2PI = float(np.log(2.0 * np.pi))

_CACHE = {}


# ----------------------------------------------------------------------------
# Host math: shared sequential recursion + block transfer operators (float64)
# ----------------------------------------------------------------------------

def _shared_recursion(P0, A, Q, H, R):
    A = A.astype(np.float64)
    Q = Q.astype(np.float64)
    H = H.astype(np.float64)
    R = R.astype(np.float64)
    P = P0.astype(np.float64)
    Pf = np.empty((T, D, D))
    Kk = np.empty((T, D, E))
    Gg = np.empty((T, D, D))
    Us = np.empty((T, E, E))
    cc = np.empty((T,))
    I = np.eye(D)
    for t in range(T):
        Pp = A @ P @ A.T + Q
        S = H @ Pp @ H.T + R
        L = np.linalg.cholesky(S)
        Linv = np.linalg.inv(L)
        Sinv = Linv.T @ Linv
        K = Pp @ H.T @ Sinv
        Pft = Pp - K @ S @ K.T
        Pf[t] = Pft
        Kk[t] = K
        Gg[t] = (I - K @ H) @ A
        Us[t] = Linv / np.sqrt(2.0)
        cc[t] = np.sum(np.log(np.diag(L))) + 0.5 * E * LN2PI
        P = Pft
    return {"Pf": Pf, "K": Kk, "G": Gg, "U": Us, "c": cc, "A": A, "H": H}


def _block_operators(shared):
    """Per-block operators: Mstack = DD@m0 + EE@Y, Zstack = FF@m0 + GG@Y."""
    G, K, U, A, H = (shared[k] for k in ("G", "K", "U", "A", "H"))
    UHA = np.einsum("tij,jk,kl->til", U, H, A)  # [T,E,D]

    DD = np.zeros((NBLK, LBLK * D, D))
    EE = np.zeros((NBLK, LBLK * D, LBLK * E))
    FF = np.zeros((NBLK, LBLK * E, D))
    GGm = np.zeros((NBLK, LBLK * E, LBLK * E))

    for j in range(NBLK):
        t0 = j * LBLK
        Dprev = np.eye(D)
        CK = {}  # k -> Phi_{i,k} @ K_{t0+k}
        for i in range(1, LBLK + 1):
            t = t0 + i - 1
            FF[j, (i - 1) * E : i * E, :] = -UHA[t] @ Dprev
            for k, v in CK.items():
                GGm[j, (i - 1) * E : i * E, (k - 1) * E : k * E] = -UHA[t] @ v
            GGm[j, (i - 1) * E : i * E, (i - 1) * E : i * E] += U[t]
            Dcur = G[t] @ Dprev
            for k in list(CK):
                CK[k] = G[t] @ CK[k]
            CK[i] = K[t].copy()
            DD[j, (i - 1) * D : i * D, :] = Dcur
            for k, v in CK.items():
                EE[j, (i - 1) * D : i * D, (k - 1) * E : k * E] = v
            Dprev = Dcur
    # Permute row-blocks of the means stack so the carry row-block (i=LBLK)
    # sits on partitions 0..D-1: compute engines cannot copy across
    # partitions, so the PSUM->SBUF carry copy must be partition-aligned.
    perm = [LBLK - 1] + list(range(LBLK - 1))  # new rb 0 <- i1=15, rb k <- i1=k-1
    DD = DD.reshape(NBLK, LBLK, D, D)[:, perm].reshape(NBLK, LBLK * D, D)
    EE = EE.reshape(NBLK, LBLK, D, LBLK * E)[:, perm].reshape(
        NBLK, LBLK * D, LBLK * E
    )
    return DD, EE, FF, GGm


# ----------------------------------------------------------------------------
# Device program
# ----------------------------------------------------------------------------

def _build_nc():
    import concourse.bacc as bacc
    import concourse.tile as tile
    from concourse import mybir

    f32 = mybir.dt.float32
    f32r = mybir.dt.float32r
    nc = bacc.Bacc("TRN2", target_bir_lowering=False, debug=False)

    KR = D + LBLK * E  # 72: carry rows stacked on top of the block's Y rows
    yb_d = nc.dram_tensor("ybig", [KR, NBLK * BC], f32, kind="ExternalInput")
    deT_d = nc.dram_tensor("deT", [KR, NBLK * LBLK * D], f32, kind="ExternalInput")
    fgT_d = nc.dram_tensor("fgT", [KR, NBLK * LBLK * E], f32, kind="ExternalInput")
    pf_d = nc.dram_tensor("pf", [128, T * D * D // 128], f32, kind="ExternalInput")
    ones_d = nc.dram_tensor("ones", [LBLK * E, 1], f32, kind="ExternalInput")
    c_d = nc.dram_tensor("cconst", [1, 1], f32, kind="ExternalInput")

    covs_d = nc.dram_tensor("covs", [BC, T * D * D], f32, kind="ExternalOutput")
    mst_d = nc.dram_tensor("mstage", [LBLK * D, NBLK * BC], f32, kind="ExternalOutput")
    ll_d = nc.dram_tensor("ll", [1, BC], f32, kind="ExternalOutput")

    PFREE = T * D * D // 128  # 256

    with tile.TileContext(nc) as tc:
        with (
            tc.tile_pool(name="const", bufs=1) as cpool,
            tc.tile_pool(name="work", bufs=4) as wpool,
            tc.tile_pool(name="psum", bufs=3, space="PSUM") as ppool,
            tc.tile_pool(name="psll", bufs=1, space="PSUM") as llpool,
        ):
            # pf loads first on the sync HWDGE ring so the covs broadcast
            # writes (the memory-bound bulk) start immediately behind it;
            # all other inputs load via the gpsimd SWDGE ring in parallel.
            pf_t = cpool.tile([128, PFREE], f32)
            nc.sync.dma_start(pf_t[:], pf_d[:])
            # ybig rows 0..D-1 hold the per-block carry (block 0's carry =
            # m0, preloaded from host; later blocks written on-device);
            # rows D.. hold the static stacked emissions.
            yb_t = cpool.tile([KR, NBLK * BC], f32)
            nc.gpsimd.dma_start(yb_t[:], yb_d[:])
            deT_t = cpool.tile([KR, NBLK * LBLK * D], f32)
            nc.gpsimd.dma_start(deT_t[:], deT_d[:])
            fgT_t = cpool.tile([KR, NBLK * LBLK * E], f32)
            nc.gpsimd.dma_start(fgT_t[:], fgT_d[:])
            ones_t = cpool.tile([LBLK * E, 1], f32)
            nc.gpsimd.dma_start(ones_t[:], ones_d[:])
            c_t = cpool.tile([1, 1], f32)
            nc.gpsimd.dma_start(c_t[:], c_d[:])
            # all filtered means accumulate here; written out in one DMA
            msb_t = cpool.tile([LBLK * D, NBLK * BC], f32)

            # Both matmul stacks stay exact fp32. Only the final ones-
            # reduction of the (positive) squared z runs in FP32r — input
            # rounding there perturbs ll by ~1e-6 relative. FP32r operands
            # must come from instructions that round to FP32r (the ACT
            # square writes zsq as f32r; ones converted here).
            onesr_t = cpool.tile([LBLK * E, 1], f32r)
            nc.vector.tensor_copy(onesr_t[:], ones_t[:])

            ll_ps = llpool.tile([1, BC], f32)

            import os

            salt = int(os.environ.get("KF_SALT", "0"))
            if salt:
                # force a unique NEFF so terminal-side NTFF capture sees a
                # fresh executable (profiling-only knob, no effect on math)
                salt_t = cpool.tile([1, salt], f32)
                nc.vector.memset(salt_t[:], 0.0)

            # Broadcast-write the shared covariance trajectory to all batch
            # rows: 8 DMAs x 32 rows x 128KB (the memory-bound bulk).
            NGRP = 8
            GRP = BC // NGRP
            for g in range(NGRP):
                out_ap = covs_d[g * GRP : (g + 1) * GRP, :].rearrange(
                    "b (p f) -> p b f", p=128
                )
                nc.sync.dma_start(
                    out_ap, pf_t[:, None, :].broadcast_to([128, GRP, PFREE])
                )

            pending_zsq = None  # defer ll matmul one block for PE slack
            for j in range(NBLK):
                m_ps = ppool.tile([LBLK * D, BC], f32, tag="mps")
                z_ps = ppool.tile([LBLK * E, BC], f32, tag="zps")
                nc.tensor.matmul(
                    m_ps[:],
                    deT_t[:, j * 128 : (j + 1) * 128],
                    yb_t[:, j * BC : (j + 1) * BC],
                    start=True,
                    stop=True,
                )
                nc.tensor.matmul(
                    z_ps[:],
                    fgT_t[:, j * 64 : (j + 1) * 64],
                    yb_t[:, j * BC : (j + 1) * BC],
                    start=True,
                    stop=True,
                )
                if pending_zsq is not None:
                    nc.tensor.matmul(
                        ll_ps[:],
                        onesr_t[:],
                        pending_zsq[:],
                        start=(j == 1),
                        stop=False,
                    )
                if j + 1 < NBLK:
                    # next block's carry rows (the sequential chain)
                    nc.vector.tensor_copy(
                        yb_t[:D, (j + 1) * BC : (j + 2) * BC], m_ps[:D, :]
                    )
                nc.vector.tensor_copy(msb_t[:, j * BC : (j + 1) * BC], m_ps[:])
                zsq = wpool.tile([LBLK * E, BC], f32r, tag="zsq")
                nc.scalar.square(zsq[:], z_ps[:])
                pending_zsq = zsq

            nc.tensor.matmul(
                ll_ps[:],
                onesr_t[:],
                pending_zsq[:],
                start=False,
                stop=True,
            )
            ll_sb = wpool.tile([1, BC], f32, tag="llsb")
            # ll = -acc - C
            nc.vector.tensor_scalar(
                ll_sb[:],
                ll_ps[:],
                -1.0,
                c_t[0:1, :],
                mybir.AluOpType.mult,
                mybir.AluOpType.subtract,
            )
            nc.scalar.dma_start(ll_d[:], ll_sb[:])
            nc.scalar.dma_start(mst_d[:], msb_t[:])

    nc.compile()
    return nc


def _get_nc():
    if "nc" not in _CACHE:
        _CACHE["nc"] = _build_nc()
    return _CACHE["nc"]


# ----------------------------------------------------------------------------
# Host wrapper
# ----------------------------------------------------------------------------

def _prepare_shared_inputs(P0_0, A, Q, H, R):
    shared = _shared_recursion(P0_0, A, Q, H, R)
    DD, EE, FF, GGm = _block_operators(shared)
    f = np.float32
    dT = DD.transpose(2, 0, 1).reshape(D, NBLK * 128)
    eT = EE.transpose(2, 0, 1).reshape(64, NBLK * 128)
    fT = FF.transpose(2, 0, 1).reshape(D, NBLK * 64)
    gT = GGm.transpose(2, 0, 1).reshape(64, NBLK * 64)
    deT = np.ascontiguousarray(np.concatenate([dT, eT], axis=0), f)
    fgT = np.ascontiguousarray(np.concatenate([fT, gT], axis=0), f)
    pf = np.ascontiguousarray(shared["Pf"].reshape(-1).reshape(128, -1), f)
    ones = np.ones((64, 1), f)
    cconst = np.array([[np.sum(shared["c"])]], f)
    pf_full = shared["Pf"].astype(f)  # [T, D, D]
    return dict(deT=deT, fgT=fgT, pf=pf, ones=ones, cconst=cconst), pf_full


def _numpy_fallback(emissions, m0, P0, A, Q, H, R):
    """General per-batch filter (only used if P0 is not batch-uniform)."""
    Bn = emissions.shape[0]
    A64, Q64, H64, R64 = (x.astype(np.float64) for x in (A, Q, H, R))
    m = m0.astype(np.float64)
    P = P0.astype(np.float64)
    lls = np.zeros(Bn)
    means = np.empty((Bn, T, D))
    covs = np.empty((Bn, T, D, D))
    for t in range(T):
        y = emissions[:, t, :].astype(np.float64)
        mp = m @ A64.T
        Pp = np.einsum("ij,bjk,lk->bil", A64, P, A64) + Q64
        mu = mp @ H64.T
        S = np.einsum("ij,bjk,lk->bil", H64, Pp, H64) + R64
        r = y - mu
        L = np.linalg.cholesky(S)
        z = np.linalg.solve(L, r[..., None])[..., 0]
        lls += (
            -0.5 * np.sum(z * z, axis=-1)
            - np.sum(np.log(np.diagonal(L, axis1=-2, axis2=-1)), axis=-1)
            - 0.5 * E * LN2PI
        )
        HP = np.einsum("ij,bjk->bik", H64, Pp)
        Kt = np.swapaxes(np.linalg.solve(S, HP), -1, -2)
        m = mp + np.einsum("bij,bj->bi", Kt, r)
        P = Pp - np.einsum("bij,bjk,blk->bil", Kt, S, Kt)
        means[:, t] = m
        covs[:, t] = P
    return (
        lls.astype(np.float32),
        means.astype(np.float32),
        covs.astype(np.float32),
    )


def kernel(emissions, m0, P0, A, Q, H, R):
    emissions = np.asarray(emissions, np.float32)
    m0 = np.asarray(m0, np.float32)
    P0 = np.asarray(P0, np.float32)
    A = np.asarray(A, np.float32)
    Q = np.asarray(Q, np.float32)
    H = np.asarray(H, np.float32)
    R = np.asarray(R, np.float32)

    if emissions.shape != (B, T, E) or not (P0 == P0[0]).all():
        return _numpy_fallback(emissions, m0, P0, A, Q, H, R)

    from concourse.bass_utils import run_bass_kernel_spmd

    shared_ins, _pf_full = _prepare_shared_inputs(P0[0], A, Q, H, R)

    in_maps = make_in_maps(emissions, m0, shared_ins)
    nc = _get_nc()
    res = run_bass_kernel_spmd(nc, in_maps, core_ids=list(range(NCORES))).results
    return gather(res)


def make_in_maps(emissions, m0, shared_ins):
    in_maps = []
    for c in range(NCORES):
        sl = slice(c * BC, (c + 1) * BC)
        em = np.asarray(emissions[sl], np.float32)  # [BC, T, E]
        ybig = np.zeros((D + LBLK * E, NBLK * BC), np.float32)
        ybig[D:] = (
            em.reshape(BC, NBLK, LBLK, E).transpose(2, 3, 1, 0).reshape(64, NBLK * BC)
        )
        ybig[:D, :BC] = np.asarray(m0[sl], np.float32).T  # block-0 carry
        in_maps.append({"ybig": ybig, **shared_ins})
    return in_maps


# inverse of the row-block permutation applied in _block_operators
_PERM_INV = list(range(1, LBLK)) + [0]


def gather(res):
    ll = np.empty((B,), np.float32)
    means = np.empty((B, T, D), np.float32)
    covs = np.empty((B, T, D, D), np.float32)
    for c in range(NCORES):
        sl = slice(c * BC, (c + 1) * BC)
        ll[sl] = res[c]["ll"][0]
        # mstage is [LBLK*D, NBLK*BC] with row-blocks in carry-permuted order
        means[sl] = (
            res[c]["mstage"]
            .reshape(LBLK, D, NBLK, BC)[_PERM_INV]
            .transpose(3, 2, 0, 1)
            .reshape(BC, T, D)
        )
        covs[sl] = res[c]["covs"].reshape(BC, T, D, D)
    return ll, means, covs


# revision 35
# speedup vs baseline: 1.0023x; 1.0023x over previous
"""Trainium2 Bass kernel for the batched Kalman filter problem.

Problem: emissions [2048, 512, 4], m0 [2048, 8], P0 [2048, 8, 8] (identical
across batch in the reference setup), A/Q [8,8], H [4,8], R [4,4].
Outputs: marginal_log_likelihood [2048], filtered_means [2048, 512, 8],
filtered_covariances [2048, 512, 8, 8].

Strategy
--------
P0 is identical for every batch row, so the covariance/gain recursion
(Pp = A P A' + Q, S = H Pp H' + R, K, Pf) is batch-independent: the filtered
covariances are one shared [T, D, D] trajectory and the per-batch work is

    means:  m_t = G_t m_{t-1} + K_t y_t      (time-varying linear recurrence)
    loglik: ll(b) = -sum_t ||z_t||^2/2 - C,  z_t = L_t^{-1}(y_t - H A m_{t-1})

with G_t, K_t, L_t shared. The tiny sequential T-step recursion of 8x8
matrices runs on host in float64; everything O(B*T) runs on device.

Time is chunked into blocks of 16 steps. Within a block, the stacked means
[16*8, B] and stacked whitened innovations [16*4, B] are linear in
(m_blockstart, y_block), so each is exactly two TensorEngine matmuls with
host-precomputed transfer operators. The sequential dependency is only the
[8, B] carry between blocks. Log-likelihood accumulates in PSUM via a
ones-vector matmul over squared z. The shared covariance trajectory is
broadcast-written from SBUF to every batch row of the output (the memory-
bound bulk: ~32 MB per core).

Batch 2048 is sharded 8 ways (pure data parallel, 256 rows/core); each core
runs the identical program on its shard.
"""

import numpy as np

B, T, D, E = 2048, 512, 8, 4
NCORES = 8
BC = B // NCORES  # 256 batch rows per core
LBLK = 16
NBLK = T // LBLK  # 32
LN2PI = float(np.log(2.0 * np.pi))

_CACHE = {}


# ----------------------------------------------------------------------------
# Host math: shared sequential recursion + block transfer operators (float64)
# ----------------------------------------------------------------------------

def _shared_recursion(P0, A, Q, H, R):
    A = A.astype(np.float64)
    Q = Q.astype(np.float64)
    H = H.astype(np.float64)
    R = R.astype(np.float64)
    P = P0.astype(np.float64)
    Pf = np.empty((T, D, D))
    Kk = np.empty((T, D, E))
    Gg = np.empty((T, D, D))
    Us = np.empty((T, E, E))
    cc = np.empty((T,))
    I = np.eye(D)
    for t in range(T):
        Pp = A @ P @ A.T + Q
        S = H @ Pp @ H.T + R
        L = np.linalg.cholesky(S)
        Linv = np.linalg.inv(L)
        Sinv = Linv.T @ Linv
        K = Pp @ H.T @ Sinv
        Pft = Pp - K @ S @ K.T
        Pf[t] = Pft
        Kk[t] = K
        Gg[t] = (I - K @ H) @ A
        Us[t] = Linv / np.sqrt(2.0)
        cc[t] = np.sum(np.log(np.diag(L))) + 0.5 * E * LN2PI
        P = Pft
    return {"Pf": Pf, "K": Kk, "G": Gg, "U": Us, "c": cc, "A": A, "H": H}


def _block_operators(shared):
    """Per-block operators: Mstack = DD@m0 + EE@Y, Zstack = FF@m0 + GG@Y."""
    G, K, U, A, H = (shared[k] for k in ("G", "K", "U", "A", "H"))
    UHA = np.einsum("tij,jk,kl->til", U, H, A)  # [T,E,D]

    DD = np.zeros((NBLK, LBLK * D, D))
    EE = np.zeros((NBLK, LBLK * D, LBLK * E))
    FF = np.zeros((NBLK, LBLK * E, D))
    GGm = np.zeros((NBLK, LBLK * E, LBLK * E))

    for j in range(NBLK):
        t0 = j * LBLK
        Dprev = np.eye(D)
        CK = {}  # k -> Phi_{i,k} @ K_{t0+k}
        for i in range(1, LBLK + 1):
            t = t0 + i - 1
            FF[j, (i - 1) * E : i * E, :] = -UHA[t] @ Dprev
            for k, v in CK.items():
                GGm[j, (i - 1) * E : i * E, (k - 1) * E : k * E] = -UHA[t] @ v
            GGm[j, (i - 1) * E : i * E, (i - 1) * E : i * E] += U[t]
            Dcur = G[t] @ Dprev
            for k in list(CK):
                CK[k] = G[t] @ CK[k]
            CK[i] = K[t].copy()
            DD[j, (i - 1) * D : i * D, :] = Dcur
            for k, v in CK.items():
                EE[j, (i - 1) * D : i * D, (k - 1) * E : k * E] = v
            Dprev = Dcur
    # Permute row-blocks of the means stack so the carry row-block (i=LBLK)
    # sits on partitions 0..D-1: compute engines cannot copy across
    # partitions, so the PSUM->SBUF carry copy must be partition-aligned.
    perm = [LBLK - 1] + list(range(LBLK - 1))  # new rb 0 <- i1=15, rb k <- i1=k-1
    DD = DD.reshape(NBLK, LBLK, D, D)[:, perm].reshape(NBLK, LBLK * D, D)
    EE = EE.reshape(NBLK, LBLK, D, LBLK * E)[:, perm].reshape(
        NBLK, LBLK * D, LBLK * E
    )
    return DD, EE, FF, GGm


# ----------------------------------------------------------------------------
# Device program
# ----------------------------------------------------------------------------

def _build_nc():
    import concourse.bacc as bacc
    import concourse.tile as tile
    from concourse import mybir

    f32 = mybir.dt.float32
    f32r = mybir.dt.float32r
    nc = bacc.Bacc("TRN2", target_bir_lowering=False, debug=False)

    KR = D + LBLK * E  # 72: carry rows stacked on top of the block's Y rows
    yb_d = nc.dram_tensor("ybig", [KR, NBLK * BC], f32, kind="ExternalInput")
    deT_d = nc.dram_tensor("deT", [KR, NBLK * LBLK * D], f32, kind="ExternalInput")
    fgT_d = nc.dram_tensor("fgT", [KR, NBLK * LBLK * E], f32, kind="ExternalInput")
    pf_d = nc.dram_tensor("pf", [128, T * D * D // 128], f32, kind="ExternalInput")
    ones_d = nc.dram_tensor("ones", [LBLK * E, 1], f32, kind="ExternalInput")
    c_d = nc.dram_tensor("cconst", [1, 1], f32, kind="ExternalInput")

    covs_d = nc.dram_tensor("covs", [BC, T * D * D], f32, kind="ExternalOutput")
    mst_d = nc.dram_tensor("mstage", [LBLK * D, NBLK * BC], f32, kind="ExternalOutput")
    ll_d = nc.dram_tensor("ll", [1, BC], f32, kind="ExternalOutput")

    PFREE = T * D * D // 128  # 256

    with tile.TileContext(nc) as tc:
        with (
            tc.tile_pool(name="const", bufs=1) as cpool,
            tc.tile_pool(name="work", bufs=4) as wpool,
            tc.tile_pool(name="psum", bufs=3, space="PSUM") as ppool,
            tc.tile_pool(name="psll", bufs=1, space="PSUM") as llpool,
        ):
            # pf loads first on the sync HWDGE ring so the covs broadcast
            # writes (the memory-bound bulk) start immediately behind it;
            # all other inputs load via the gpsimd SWDGE ring in parallel.
            pf_t = cpool.tile([128, PFREE], f32)
            nc.sync.dma_start(pf_t[:], pf_d[:])
            # ybig rows 0..D-1 hold the per-block carry (block 0's carry =
            # m0, preloaded from host; later blocks written on-device);
            # rows D.. hold the static stacked emissions.
            yb_t = cpool.tile([KR, NBLK * BC], f32)
            nc.gpsimd.dma_start(yb_t[:], yb_d[:])
            deT_t = cpool.tile([KR, NBLK * LBLK * D], f32)
            nc.gpsimd.dma_start(deT_t[:], deT_d[:])
            fgT_t = cpool.tile([KR, NBLK * LBLK * E], f32)
            nc.gpsimd.dma_start(fgT_t[:], fgT_d[:])
            ones_t = cpool.tile([LBLK * E, 1], f32)
            nc.gpsimd.dma_start(ones_t[:], ones_d[:])
            c_t = cpool.tile([1, 1], f32)
            nc.gpsimd.dma_start(c_t[:], c_d[:])
            # all filtered means accumulate here; written out in one DMA
            msb_t = cpool.tile([LBLK * D, NBLK * BC], f32)

            # Both matmul stacks stay exact fp32. Only the final ones-
            # reduction of the (positive) squared z runs in FP32r — input
            # rounding there perturbs ll by ~1e-6 relative. FP32r operands
            # must come from instructions that round to FP32r (the ACT
            # square writes zsq as f32r; ones converted here).
            onesr_t = cpool.tile([LBLK * E, 1], f32r)
            nc.vector.tensor_copy(onesr_t[:], ones_t[:])

            ll_ps = llpool.tile([1, BC], f32)

            # Broadcast-write the shared covariance trajectory to all batch
            # rows: 8 DMAs x 32 rows x 128KB (the memory-bound bulk).
            NGRP = 8
            GRP = BC // NGRP
            for g in range(NGRP):
                out_ap = covs_d[g * GRP : (g + 1) * GRP, :].rearrange(
                    "b (p f) -> p b f", p=128
                )
                nc.sync.dma_start(
                    out_ap, pf_t[:, None, :].broadcast_to([128, GRP, PFREE])
                )

            pending_zsq = None  # defer ll matmul one block for PE slack
            for j in range(NBLK):
                m_ps = ppool.tile([LBLK * D, BC], f32, tag="mps")
                z_ps = ppool.tile([LBLK * E, BC], f32, tag="zps")
                nc.tensor.matmul(
                    m_ps[:],
                    deT_t[:, j * 128 : (j + 1) * 128],
                    yb_t[:, j * BC : (j + 1) * BC],
                    start=True,
                    stop=True,
                )
                nc.tensor.matmul(
                    z_ps[:],
                    fgT_t[:, j * 64 : (j + 1) * 64],
                    yb_t[:, j * BC : (j + 1) * BC],
                    start=True,
                    stop=True,
                )
                if pending_zsq is not None:
                    zr, rr = pending_zsq
                    nc.tensor.matmul(
                        ll_ps[:], onesr_t[:], zr[:], start=(j == 1), stop=False
                    )
                    nc.tensor.matmul(
                        ll_ps[:], onesr_t[:], rr[:], start=False, stop=False
                    )
                if j + 1 < NBLK:
                    # next block's carry rows (the sequential chain)
                    nc.vector.tensor_copy(
                        yb_t[:D, (j + 1) * BC : (j + 2) * BC], m_ps[:D, :]
                    )
                nc.vector.tensor_copy(msb_t[:, j * BC : (j + 1) * BC], m_ps[:])
                # f32-exact ll despite the f32r ones-reduction: accumulate
                # the rounded squares plus the exact rounding residual
                zsq = wpool.tile([LBLK * E, BC], f32, tag="zsq")
                nc.scalar.square(zsq[:], z_ps[:])
                zsqr = wpool.tile([LBLK * E, BC], f32r, tag="zsqr")
                nc.vector.tensor_copy(zsqr[:], zsq[:])
                resid = wpool.tile([LBLK * E, BC], f32r, tag="resid")
                nc.vector.tensor_sub(resid[:], zsq[:], zsqr[:].bitcast(f32))
                pending_zsq = (zsqr, resid)

            zr, rr = pending_zsq
            nc.tensor.matmul(ll_ps[:], onesr_t[:], zr[:], start=False, stop=False)
            nc.tensor.matmul(ll_ps[:], onesr_t[:], rr[:], start=False, stop=True)
            ll_sb = wpool.tile([1, BC], f32, tag="llsb")
            # ll = -acc - C
            nc.vector.tensor_scalar(
                ll_sb[:],
                ll_ps[:],
                -1.0,
                c_t[0:1, :],
                mybir.AluOpType.mult,
                mybir.AluOpType.subtract,
            )
            nc.scalar.dma_start(ll_d[:], ll_sb[:])
            nc.scalar.dma_start(mst_d[:], msb_t[:])

    nc.compile()
    return nc


def _get_nc():
    if "nc" not in _CACHE:
        _CACHE["nc"] = _build_nc()
    return _CACHE["nc"]


# ----------------------------------------------------------------------------
# Host wrapper
# ----------------------------------------------------------------------------

def _prepare_shared_inputs(P0_0, A, Q, H, R):
    shared = _shared_recursion(P0_0, A, Q, H, R)
    DD, EE, FF, GGm = _block_operators(shared)
    f = np.float32
    dT = DD.transpose(2, 0, 1).reshape(D, NBLK * 128)
    eT = EE.transpose(2, 0, 1).reshape(64, NBLK * 128)
    fT = FF.transpose(2, 0, 1).reshape(D, NBLK * 64)
    gT = GGm.transpose(2, 0, 1).reshape(64, NBLK * 64)
    deT = np.ascontiguousarray(np.concatenate([dT, eT], axis=0), f)
    fgT = np.ascontiguousarray(np.concatenate([fT, gT], axis=0), f)
    pf = np.ascontiguousarray(shared["Pf"].reshape(-1).reshape(128, -1), f)
    ones = np.ones((64, 1), f)
    cconst = np.array([[np.sum(shared["c"])]], f)
    pf_full = shared["Pf"].astype(f)  # [T, D, D]
    return dict(deT=deT, fgT=fgT, pf=pf, ones=ones, cconst=cconst), pf_full


def _numpy_fallback(emissions, m0, P0, A, Q, H, R):
    """General per-batch filter (only used if the fast-path preconditions
    fail, e.g. non-uniform P0 or unexpected shapes)."""
    Bn, T_, E_ = emissions.shape
    D_ = m0.shape[1]
    A64, Q64, H64, R64 = (x.astype(np.float64) for x in (A, Q, H, R))
    m = m0.astype(np.float64)
    P = P0.astype(np.float64)
    lls = np.zeros(Bn)
    means = np.empty((Bn, T_, D_))
    covs = np.empty((Bn, T_, D_, D_))
    for t in range(T_):
        y = emissions[:, t, :].astype(np.float64)
        mp = m @ A64.T
        Pp = np.einsum("ij,bjk,lk->bil", A64, P, A64) + Q64
        mu = mp @ H64.T
        S = np.einsum("ij,bjk,lk->bil", H64, Pp, H64) + R64
        r = y - mu
        L = np.linalg.cholesky(S)
        z = np.linalg.solve(L, r[..., None])[..., 0]
        lls += (
            -0.5 * np.sum(z * z, axis=-1)
            - np.sum(np.log(np.diagonal(L, axis1=-2, axis2=-1)), axis=-1)
            - 0.5 * E_ * LN2PI
        )
        HP = np.einsum("ij,bjk->bik", H64, Pp)
        Kt = np.swapaxes(np.linalg.solve(S, HP), -1, -2)
        m = mp + np.einsum("bij,bj->bi", Kt, r)
        P = Pp - np.einsum("bij,bjk,blk->bil", Kt, S, Kt)
        means[:, t] = m
        covs[:, t] = P
    return (
        lls.astype(np.float32),
        means.astype(np.float32),
        covs.astype(np.float32),
    )


def kernel(emissions, m0, P0, A, Q, H, R):
    emissions = np.asarray(emissions, np.float32)
    m0 = np.asarray(m0, np.float32)
    P0 = np.asarray(P0, np.float32)
    A = np.asarray(A, np.float32)
    Q = np.asarray(Q, np.float32)
    H = np.asarray(H, np.float32)
    R = np.asarray(R, np.float32)

    if (
        emissions.shape != (B, T, E)
        or m0.shape != (B, D)
        or P0.shape != (B, D, D)
        or not (P0 == P0[0]).all()
    ):
        return _numpy_fallback(emissions, m0, P0, A, Q, H, R)

    from concourse.bass_utils import run_bass_kernel_spmd

    shared_ins, _pf_full = _prepare_shared_inputs(P0[0], A, Q, H, R)

    in_maps = make_in_maps(emissions, m0, shared_ins)
    nc = _get_nc()
    res = run_bass_kernel_spmd(nc, in_maps, core_ids=list(range(NCORES))).results
    return gather(res)


def make_in_maps(emissions, m0, shared_ins):
    in_maps = []
    for c in range(NCORES):
        sl = slice(c * BC, (c + 1) * BC)
        em = np.asarray(emissions[sl], np.float32)  # [BC, T, E]
        ybig = np.zeros((D + LBLK * E, NBLK * BC), np.float32)
        ybig[D:] = (
            em.reshape(BC, NBLK, LBLK, E).transpose(2, 3, 1, 0).reshape(64, NBLK * BC)
        )
        ybig[:D, :BC] = np.asarray(m0[sl], np.float32).T  # block-0 carry
        in_maps.append({"ybig": ybig, **shared_ins})
    return in_maps


# inverse of the row-block permutation applied in _block_operators
_PERM_INV = list(range(1, LBLK)) + [0]


def gather(res):
    ll = np.empty((B,), np.float32)
    means = np.empty((B, T, D), np.float32)
    covs = np.empty((B, T, D, D), np.float32)
    for c in range(NCORES):
        sl = slice(c * BC, (c + 1) * BC)
        ll[sl] = res[c]["ll"][0]
        # mstage is [LBLK*D, NBLK*BC] with row-blocks in carry-permuted order
        means[sl] = (
            res[c]["mstage"]
            .reshape(LBLK, D, NBLK, BC)[_PERM_INV]
            .transpose(3, 2, 0, 1)
            .reshape(BC, T, D)
        )
        covs[sl] = res[c]["covs"].reshape(BC, T, D, D)
    return ll, means, covs


# revision 45
# speedup vs baseline: 1.0541x; 1.0516x over previous
"""Trainium2 Bass kernel for the batched Kalman filter problem.

Problem: emissions [2048, 512, 4], m0 [2048, 8], P0 [2048, 8, 8] (identical
across batch in the reference setup), A/Q [8,8], H [4,8], R [4,4].
Outputs: marginal_log_likelihood [2048], filtered_means [2048, 512, 8],
filtered_covariances [2048, 512, 8, 8].

Strategy
--------
P0 is identical for every batch row, so the covariance/gain recursion
(Pp = A P A' + Q, S = H Pp H' + R, K, Pf) is batch-independent: the filtered
covariances are one shared [T, D, D] trajectory and the per-batch work is

    means:  m_t = G_t m_{t-1} + K_t y_t      (time-varying linear recurrence)
    loglik: ll(b) = -sum_t ||z_t||^2/2 - C,  z_t = L_t^{-1}(y_t - H A m_{t-1})

with G_t, K_t, L_t shared. The tiny sequential T-step recursion of 8x8
matrices runs on host in float64; everything O(B*T) runs on device.

Time is chunked into blocks of 16 steps. Within a block, the stacked means
[16*8, B] and stacked whitened innovations [16*4, B] are linear in
(m_blockstart, y_block), so each is exactly two TensorEngine matmuls with
host-precomputed transfer operators. The sequential dependency is only the
[8, B] carry between blocks. Log-likelihood accumulates in PSUM via a
ones-vector matmul over squared z. The shared covariance trajectory is
broadcast-written from SBUF to every batch row of the output (the memory-
bound bulk: ~32 MB per core).

Batch 2048 is sharded 8 ways (pure data parallel, 256 rows/core); each core
runs the identical program on its shard.
"""

import numpy as np

B, T, D, E = 2048, 512, 8, 4
NCORES = 8
BC = B // NCORES  # 256 batch rows per core
LBLK = 16
NBLK = T // LBLK  # 32
LN2PI = float(np.log(2.0 * np.pi))

_CACHE = {}


# ----------------------------------------------------------------------------
# Host math: shared sequential recursion + block transfer operators (float64)
# ----------------------------------------------------------------------------

def _shared_recursion(P0, A, Q, H, R):
    A = A.astype(np.float64)
    Q = Q.astype(np.float64)
    H = H.astype(np.float64)
    R = R.astype(np.float64)
    P = P0.astype(np.float64)
    Pf = np.empty((T, D, D))
    Kk = np.empty((T, D, E))
    Gg = np.empty((T, D, D))
    Us = np.empty((T, E, E))
    cc = np.empty((T,))
    I = np.eye(D)
    for t in range(T):
        Pp = A @ P @ A.T + Q
        S = H @ Pp @ H.T + R
        L = np.linalg.cholesky(S)
        Linv = np.linalg.inv(L)
        Sinv = Linv.T @ Linv
        K = Pp @ H.T @ Sinv
        Pft = Pp - K @ S @ K.T
        Pf[t] = Pft
        Kk[t] = K
        Gg[t] = (I - K @ H) @ A
        Us[t] = Linv / np.sqrt(2.0)
        cc[t] = np.sum(np.log(np.diag(L))) + 0.5 * E * LN2PI
        P = Pft
    return {"Pf": Pf, "K": Kk, "G": Gg, "U": Us, "c": cc, "A": A, "H": H}


def _block_operators(shared):
    """Per-block operators: Mstack = DD@m0 + EE@Y, Zstack = FF@m0 + GG@Y."""
    G, K, U, A, H = (shared[k] for k in ("G", "K", "U", "A", "H"))
    UHA = np.einsum("tij,jk,kl->til", U, H, A)  # [T,E,D]

    DD = np.zeros((NBLK, LBLK * D, D))
    EE = np.zeros((NBLK, LBLK * D, LBLK * E))
    FF = np.zeros((NBLK, LBLK * E, D))
    GGm = np.zeros((NBLK, LBLK * E, LBLK * E))

    for j in range(NBLK):
        t0 = j * LBLK
        Dprev = np.eye(D)
        CK = {}  # k -> Phi_{i,k} @ K_{t0+k}
        for i in range(1, LBLK + 1):
            t = t0 + i - 1
            FF[j, (i - 1) * E : i * E, :] = -UHA[t] @ Dprev
            for k, v in CK.items():
                GGm[j, (i - 1) * E : i * E, (k - 1) * E : k * E] = -UHA[t] @ v
            GGm[j, (i - 1) * E : i * E, (i - 1) * E : i * E] += U[t]
            Dcur = G[t] @ Dprev
            for k in list(CK):
                CK[k] = G[t] @ CK[k]
            CK[i] = K[t].copy()
            DD[j, (i - 1) * D : i * D, :] = Dcur
            for k, v in CK.items():
                EE[j, (i - 1) * D : i * D, (k - 1) * E : k * E] = v
            Dprev = Dcur
    # Permute row-blocks of the means stack so the carry row-block (i=LBLK)
    # sits on partitions 0..D-1: compute engines cannot copy across
    # partitions, so the PSUM->SBUF carry copy must be partition-aligned.
    perm = [LBLK - 1] + list(range(LBLK - 1))  # new rb 0 <- i1=15, rb k <- i1=k-1
    DD = DD.reshape(NBLK, LBLK, D, D)[:, perm].reshape(NBLK, LBLK * D, D)
    EE = EE.reshape(NBLK, LBLK, D, LBLK * E)[:, perm].reshape(
        NBLK, LBLK * D, LBLK * E
    )
    return DD, EE, FF, GGm


# ----------------------------------------------------------------------------
# Device program
# ----------------------------------------------------------------------------

def _build_nc():
    import concourse.bacc as bacc
    import concourse.tile as tile
    from concourse import mybir

    f32 = mybir.dt.float32
    f32r = mybir.dt.float32r
    nc = bacc.Bacc("TRN2", target_bir_lowering=False, debug=False)

    KR = D + LBLK * E  # 72: carry rows stacked on top of the block's Y rows
    yb_d = nc.dram_tensor("ybig", [KR, NBLK * BC], f32, kind="ExternalInput")
    deT_d = nc.dram_tensor("deT", [KR, NBLK * LBLK * D], f32, kind="ExternalInput")
    fgT_d = nc.dram_tensor("fgT", [KR, NBLK * LBLK * E], f32, kind="ExternalInput")
    pf_d = nc.dram_tensor("pf", [128, T * D * D // 128], f32, kind="ExternalInput")
    ones_d = nc.dram_tensor("ones", [2 * LBLK * E, 1], f32, kind="ExternalInput")
    c_d = nc.dram_tensor("cconst", [1, 1], f32, kind="ExternalInput")

    covs_d = nc.dram_tensor("covs", [BC, T * D * D], f32, kind="ExternalOutput")
    mst_d = nc.dram_tensor("mstage", [LBLK * D, NBLK * BC], f32, kind="ExternalOutput")
    ll_d = nc.dram_tensor("ll", [1, BC], f32, kind="ExternalOutput")

    PFREE = T * D * D // 128  # 256

    with tile.TileContext(nc) as tc:
        with (
            tc.tile_pool(name="const", bufs=1) as cpool,
            tc.tile_pool(name="work", bufs=4) as wpool,
            tc.tile_pool(name="psum", bufs=3, space="PSUM") as ppool,
            tc.tile_pool(name="psll", bufs=1, space="PSUM") as llpool,
        ):
            # All inputs load on the sync HWDGE ring BEFORE the covs
            # broadcast writes: the ring is FIFO, so this guarantees the
            # compute loop starts by ~15us while the covs bulk (the
            # memory-bound 32MB) streams behind it.
            pf_t = cpool.tile([128, PFREE], f32)
            nc.sync.dma_start(pf_t[:], pf_d[:])
            # ybig rows 0..D-1 hold the per-block carry (block 0's carry =
            # m0, preloaded from host; later blocks written on-device);
            # rows D.. hold the static stacked emissions.
            yb_t = cpool.tile([KR, NBLK * BC], f32)
            nc.sync.dma_start(yb_t[:], yb_d[:])
            deT_t = cpool.tile([KR, NBLK * LBLK * D], f32)
            nc.sync.dma_start(deT_t[:], deT_d[:])
            fgT_t = cpool.tile([KR, NBLK * LBLK * E], f32)
            nc.sync.dma_start(fgT_t[:], fgT_d[:])
            ones_t = cpool.tile([2 * LBLK * E, 1], f32)
            nc.sync.dma_start(ones_t[:], ones_d[:])
            c_t = cpool.tile([1, 1], f32)
            nc.sync.dma_start(c_t[:], c_d[:])
            # all filtered means accumulate here; written out in one DMA
            msb_t = cpool.tile([LBLK * D, NBLK * BC], f32)

            # Both matmul stacks stay exact fp32. Only the final ones-
            # reduction of the squared z runs in FP32r (1 cyc/col), made
            # f32-exact by also accumulating the rounding residual: the
            # [128, BC] rhs stacks rounded squares (rows 0..63) on top of
            # the residuals (rows 64..127), reduced by one ones-vector
            # matmul. FP32r operands must come from rounding producers.
            onesr_t = cpool.tile([2 * LBLK * E, 1], f32r)
            nc.vector.tensor_copy(onesr_t[:], ones_t[:])

            ll_ps = llpool.tile([1, BC], f32)

            # Broadcast-write the shared covariance trajectory to all batch
            # rows: 8 DMAs x 32 rows x 128KB (the memory-bound bulk).
            NGRP = 8
            GRP = BC // NGRP
            for g in range(NGRP):
                out_ap = covs_d[g * GRP : (g + 1) * GRP, :].rearrange(
                    "b (p f) -> p b f", p=128
                )
                nc.sync.dma_start(
                    out_ap, pf_t[:, None, :].broadcast_to([128, GRP, PFREE])
                )

            pending_zsq = None  # defer ll matmul one block for PE slack
            for j in range(NBLK):
                m_ps = ppool.tile([LBLK * D, BC], f32, tag="mps")
                z_ps = ppool.tile([LBLK * E, BC], f32, tag="zps")
                nc.tensor.matmul(
                    m_ps[:],
                    deT_t[:, j * 128 : (j + 1) * 128],
                    yb_t[:, j * BC : (j + 1) * BC],
                    start=True,
                    stop=True,
                )
                nc.tensor.matmul(
                    z_ps[:],
                    fgT_t[:, j * 64 : (j + 1) * 64],
                    yb_t[:, j * BC : (j + 1) * BC],
                    start=True,
                    stop=True,
                )
                if pending_zsq is not None:
                    zr, rr = pending_zsq
                    nc.tensor.matmul(
                        ll_ps[:],
                        onesr_t[: LBLK * E],
                        zr[:],
                        start=(j == 1),
                        stop=False,
                    )
                    nc.tensor.matmul(
                        ll_ps[:],
                        onesr_t[: LBLK * E],
                        rr[:],
                        start=False,
                        stop=False,
                    )
                if j + 1 < NBLK:
                    # next block's carry rows (the sequential chain)
                    nc.vector.tensor_copy(
                        yb_t[:D, (j + 1) * BC : (j + 2) * BC], m_ps[:D, :]
                    )
                nc.vector.tensor_copy(msb_t[:, j * BC : (j + 1) * BC], m_ps[:])
                # f32-exact ll despite the f32r ones-reduction: accumulate
                # the rounded squares plus the exact rounding residual
                zsq = wpool.tile([LBLK * E, BC], f32, tag="zsq")
                nc.scalar.square(zsq[:], z_ps[:])
                zsqr = wpool.tile([LBLK * E, BC], f32r, tag="zsqr")
                nc.vector.tensor_copy(zsqr[:], zsq[:])
                resid = wpool.tile([LBLK * E, BC], f32r, tag="resid")
                nc.vector.tensor_sub(resid[:], zsq[:], zsqr[:].bitcast(f32))
                pending_zsq = (zsqr, resid)

            zr, rr = pending_zsq
            nc.tensor.matmul(ll_ps[:], onesr_t[:LBLK * E], zr[:], start=False, stop=False)
            nc.tensor.matmul(ll_ps[:], onesr_t[:LBLK * E], rr[:], start=False, stop=True)
            ll_sb = wpool.tile([1, BC], f32, tag="llsb")
            # ll = -acc - C
            nc.vector.tensor_scalar(
                ll_sb[:],
                ll_ps[:],
                -1.0,
                c_t[0:1, :],
                mybir.AluOpType.mult,
                mybir.AluOpType.subtract,
            )
            nc.scalar.dma_start(ll_d[:], ll_sb[:])
            nc.scalar.dma_start(mst_d[:], msb_t[:])

    nc.compile()
    return nc


def _get_nc():
    if "nc" not in _CACHE:
        _CACHE["nc"] = _build_nc()
    return _CACHE["nc"]


# ----------------------------------------------------------------------------
# Host wrapper
# ----------------------------------------------------------------------------

def _prepare_shared_inputs(P0_0, A, Q, H, R):
    shared = _shared_recursion(P0_0, A, Q, H, R)
    DD, EE, FF, GGm = _block_operators(shared)
    f = np.float32
    dT = DD.transpose(2, 0, 1).reshape(D, NBLK * 128)
    eT = EE.transpose(2, 0, 1).reshape(64, NBLK * 128)
    fT = FF.transpose(2, 0, 1).reshape(D, NBLK * 64)
    gT = GGm.transpose(2, 0, 1).reshape(64, NBLK * 64)
    deT = np.ascontiguousarray(np.concatenate([dT, eT], axis=0), f)
    fgT = np.ascontiguousarray(np.concatenate([fT, gT], axis=0), f)
    pf = np.ascontiguousarray(shared["Pf"].reshape(-1).reshape(128, -1), f)
    ones = np.ones((128, 1), f)
    cconst = np.array([[np.sum(shared["c"])]], f)
    pf_full = shared["Pf"].astype(f)  # [T, D, D]
    return dict(deT=deT, fgT=fgT, pf=pf, ones=ones, cconst=cconst), pf_full


def _numpy_fallback(emissions, m0, P0, A, Q, H, R):
    """General per-batch filter (only used if the fast-path preconditions
    fail, e.g. non-uniform P0 or unexpected shapes)."""
    Bn, T_, E_ = emissions.shape
    D_ = m0.shape[1]
    A64, Q64, H64, R64 = (x.astype(np.float64) for x in (A, Q, H, R))
    m = m0.astype(np.float64)
    P = P0.astype(np.float64)
    lls = np.zeros(Bn)
    means = np.empty((Bn, T_, D_))
    covs = np.empty((Bn, T_, D_, D_))
    for t in range(T_):
        y = emissions[:, t, :].astype(np.float64)
        mp = m @ A64.T
        Pp = np.einsum("ij,bjk,lk->bil", A64, P, A64) + Q64
        mu = mp @ H64.T
        S = np.einsum("ij,bjk,lk->bil", H64, Pp, H64) + R64
        r = y - mu
        L = np.linalg.cholesky(S)
        z = np.linalg.solve(L, r[..., None])[..., 0]
        lls += (
            -0.5 * np.sum(z * z, axis=-1)
            - np.sum(np.log(np.diagonal(L, axis1=-2, axis2=-1)), axis=-1)
            - 0.5 * E_ * LN2PI
        )
        HP = np.einsum("ij,bjk->bik", H64, Pp)
        Kt = np.swapaxes(np.linalg.solve(S, HP), -1, -2)
        m = mp + np.einsum("bij,bj->bi", Kt, r)
        P = Pp - np.einsum("bij,bjk,blk->bil", Kt, S, Kt)
        means[:, t] = m
        covs[:, t] = P
    return (
        lls.astype(np.float32),
        means.astype(np.float32),
        covs.astype(np.float32),
    )


def kernel(emissions, m0, P0, A, Q, H, R):
    emissions = np.asarray(emissions, np.float32)
    m0 = np.asarray(m0, np.float32)
    P0 = np.asarray(P0, np.float32)
    A = np.asarray(A, np.float32)
    Q = np.asarray(Q, np.float32)
    H = np.asarray(H, np.float32)
    R = np.asarray(R, np.float32)

    if (
        emissions.shape != (B, T, E)
        or m0.shape != (B, D)
        or P0.shape != (B, D, D)
        or not (P0 == P0[0]).all()
    ):
        return _numpy_fallback(emissions, m0, P0, A, Q, H, R)

    from concourse.bass_utils import run_bass_kernel_spmd

    shared_ins, _pf_full = _prepare_shared_inputs(P0[0], A, Q, H, R)

    in_maps = make_in_maps(emissions, m0, shared_ins)
    nc = _get_nc()
    res = run_bass_kernel_spmd(nc, in_maps, core_ids=list(range(NCORES))).results
    return gather(res)


def make_in_maps(emissions, m0, shared_ins):
    in_maps = []
    for c in range(NCORES):
        sl = slice(c * BC, (c + 1) * BC)
        em = np.asarray(emissions[sl], np.float32)  # [BC, T, E]
        ybig = np.zeros((D + LBLK * E, NBLK * BC), np.float32)
        ybig[D:] = (
            em.reshape(BC, NBLK, LBLK, E).transpose(2, 3, 1, 0).reshape(64, NBLK * BC)
        )
        ybig[:D, :BC] = np.asarray(m0[sl], np.float32).T  # block-0 carry
        in_maps.append({"ybig": ybig, **shared_ins})
    return in_maps


# inverse of the row-block permutation applied in _block_operators
_PERM_INV = list(range(1, LBLK)) + [0]


def gather(res):
    ll = np.empty((B,), np.float32)
    means = np.empty((B, T, D), np.float32)
    covs = np.empty((B, T, D, D), np.float32)
    for c in range(NCORES):
        sl = slice(c * BC, (c + 1) * BC)
        ll[sl] = res[c]["ll"][0]
        # mstage is [LBLK*D, NBLK*BC] with row-blocks in carry-permuted order
        means[sl] = (
            res[c]["mstage"]
            .reshape(LBLK, D, NBLK, BC)[_PERM_INV]
            .transpose(3, 2, 0, 1)
            .reshape(BC, T, D)
        )
        covs[sl] = res[c]["covs"].reshape(BC, T, D, D)
    return ll, means, covs


# revision 48
# speedup vs baseline: 1.0694x; 1.0145x over previous
"""Trainium2 Bass kernel for the batched Kalman filter problem.

Problem: emissions [2048, 512, 4], m0 [2048, 8], P0 [2048, 8, 8] (identical
across batch in the reference setup), A/Q [8,8], H [4,8], R [4,4].
Outputs: marginal_log_likelihood [2048], filtered_means [2048, 512, 8],
filtered_covariances [2048, 512, 8, 8].

Strategy
--------
P0 is identical for every batch row, so the covariance/gain recursion
(Pp = A P A' + Q, S = H Pp H' + R, K, Pf) is batch-independent: the filtered
covariances are one shared [T, D, D] trajectory and the per-batch work is

    means:  m_t = G_t m_{t-1} + K_t y_t      (time-varying linear recurrence)
    loglik: ll(b) = -sum_t ||z_t||^2/2 - C,  z_t = L_t^{-1}(y_t - H A m_{t-1})

with G_t, K_t, L_t shared. The tiny sequential T-step recursion of 8x8
matrices runs on host in float64; everything O(B*T) runs on device.

Time is chunked into blocks of 16 steps. Within a block, the stacked means
[16*8, B] and stacked whitened innovations [16*4, B] are linear in
(m_blockstart, y_block), so each is exactly two TensorEngine matmuls with
host-precomputed transfer operators. The sequential dependency is only the
[8, B] carry between blocks. Log-likelihood accumulates in PSUM via a
ones-vector matmul over squared z. The shared covariance trajectory is
broadcast-written from SBUF to every batch row of the output (the memory-
bound bulk: ~32 MB per core).

Batch 2048 is sharded 8 ways (pure data parallel, 256 rows/core); each core
runs the identical program on its shard.
"""

import numpy as np

B, T, D, E = 2048, 512, 8, 4
NCORES = 8
BC = B // NCORES  # 256 batch rows per core
LBLK = 16
NBLK = T // LBLK  # 32
LN2PI = float(np.log(2.0 * np.pi))

_CACHE = {}


# ----------------------------------------------------------------------------
# Host math: shared sequential recursion + block transfer operators (float64)
# ----------------------------------------------------------------------------

def _shared_recursion(P0, A, Q, H, R):
    A = A.astype(np.float64)
    Q = Q.astype(np.float64)
    H = H.astype(np.float64)
    R = R.astype(np.float64)
    P = P0.astype(np.float64)
    Pf = np.empty((T, D, D))
    Kk = np.empty((T, D, E))
    Gg = np.empty((T, D, D))
    Us = np.empty((T, E, E))
    cc = np.empty((T,))
    I = np.eye(D)
    for t in range(T):
        Pp = A @ P @ A.T + Q
        S = H @ Pp @ H.T + R
        L = np.linalg.cholesky(S)
        Linv = np.linalg.inv(L)
        Sinv = Linv.T @ Linv
        K = Pp @ H.T @ Sinv
        Pft = Pp - K @ S @ K.T
        Pf[t] = Pft
        Kk[t] = K
        Gg[t] = (I - K @ H) @ A
        Us[t] = Linv / np.sqrt(2.0)
        cc[t] = np.sum(np.log(np.diag(L))) + 0.5 * E * LN2PI
        P = Pft
    return {"Pf": Pf, "K": Kk, "G": Gg, "U": Us, "c": cc, "A": A, "H": H}


def _block_operators(shared):
    """Per-block operators: Mstack = DD@m0 + EE@Y, Zstack = FF@m0 + GG@Y."""
    G, K, U, A, H = (shared[k] for k in ("G", "K", "U", "A", "H"))
    UHA = np.einsum("tij,jk,kl->til", U, H, A)  # [T,E,D]

    DD = np.zeros((NBLK, LBLK * D, D))
    EE = np.zeros((NBLK, LBLK * D, LBLK * E))
    FF = np.zeros((NBLK, LBLK * E, D))
    GGm = np.zeros((NBLK, LBLK * E, LBLK * E))

    for j in range(NBLK):
        t0 = j * LBLK
        Dprev = np.eye(D)
        CK = {}  # k -> Phi_{i,k} @ K_{t0+k}
        for i in range(1, LBLK + 1):
            t = t0 + i - 1
            FF[j, (i - 1) * E : i * E, :] = -UHA[t] @ Dprev
            for k, v in CK.items():
                GGm[j, (i - 1) * E : i * E, (k - 1) * E : k * E] = -UHA[t] @ v
            GGm[j, (i - 1) * E : i * E, (i - 1) * E : i * E] += U[t]
            Dcur = G[t] @ Dprev
            for k in list(CK):
                CK[k] = G[t] @ CK[k]
            CK[i] = K[t].copy()
            DD[j, (i - 1) * D : i * D, :] = Dcur
            for k, v in CK.items():
                EE[j, (i - 1) * D : i * D, (k - 1) * E : k * E] = v
            Dprev = Dcur
    # Permute row-blocks of the means stack so the carry row-block (i=LBLK)
    # sits on partitions 0..D-1: compute engines cannot copy across
    # partitions, so the PSUM->SBUF carry copy must be partition-aligned.
    perm = [LBLK - 1] + list(range(LBLK - 1))  # new rb 0 <- i1=15, rb k <- i1=k-1
    DD = DD.reshape(NBLK, LBLK, D, D)[:, perm].reshape(NBLK, LBLK * D, D)
    EE = EE.reshape(NBLK, LBLK, D, LBLK * E)[:, perm].reshape(
        NBLK, LBLK * D, LBLK * E
    )
    return DD, EE, FF, GGm


# ----------------------------------------------------------------------------
# Device program
# ----------------------------------------------------------------------------

def _build_nc():
    import concourse.bacc as bacc
    import concourse.tile as tile
    from concourse import mybir

    f32 = mybir.dt.float32
    f32r = mybir.dt.float32r
    nc = bacc.Bacc("TRN2", target_bir_lowering=False, debug=False)

    KR = D + LBLK * E  # 72: carry rows stacked on top of the block's Y rows
    yb_d = nc.dram_tensor("ybig", [KR, NBLK * BC], f32, kind="ExternalInput")
    deT_d = nc.dram_tensor("deT", [KR, NBLK * LBLK * D], f32, kind="ExternalInput")
    fgT_d = nc.dram_tensor("fgT", [KR, NBLK * LBLK * E], f32, kind="ExternalInput")
    pf_d = nc.dram_tensor("pf", [128, T * D * D // 128], f32, kind="ExternalInput")
    ones_d = nc.dram_tensor("ones", [2 * LBLK * E, 1], f32, kind="ExternalInput")
    c_d = nc.dram_tensor("cconst", [1, 1], f32, kind="ExternalInput")

    covs_d = nc.dram_tensor("covs", [BC, T * D * D], f32, kind="ExternalOutput")
    mst_d = nc.dram_tensor("mstage", [LBLK * D, NBLK * BC], f32, kind="ExternalOutput")
    ll_d = nc.dram_tensor("ll", [1, BC], f32, kind="ExternalOutput")

    PFREE = T * D * D // 128  # 256

    with tile.TileContext(nc) as tc:
        with (
            tc.tile_pool(name="const", bufs=1) as cpool,
            tc.tile_pool(name="work", bufs=4) as wpool,
            tc.tile_pool(name="psum", bufs=3, space="PSUM") as ppool,
            tc.tile_pool(name="psll", bufs=1, space="PSUM") as llpool,
        ):
            # All inputs load on the sync HWDGE ring BEFORE the covs
            # broadcast writes: the ring is FIFO, so this guarantees the
            # compute loop starts by ~15us while the covs bulk (the
            # memory-bound 32MB) streams behind it.
            pf_t = cpool.tile([128, PFREE], f32)
            nc.sync.dma_start(pf_t[:], pf_d[:])
            # ybig rows 0..D-1 hold the per-block carry (block 0's carry =
            # m0, preloaded from host; later blocks written on-device);
            # rows D.. hold the static stacked emissions. The first 8
            # blocks' slices load first so the compute loop starts while
            # the rest (and the covs bulk) stream in behind.
            NHEAD = 8
            yb_t = cpool.tile([KR, NBLK * BC], f32)
            deT_t = cpool.tile([KR, NBLK * LBLK * D], f32)
            fgT_t = cpool.tile([KR, NBLK * LBLK * E], f32)
            nc.sync.dma_start(
                yb_t[:, : NHEAD * BC], yb_d[:, : NHEAD * BC]
            )
            nc.sync.dma_start(
                deT_t[:, : NHEAD * 128], deT_d[:, : NHEAD * 128]
            )
            nc.sync.dma_start(
                fgT_t[:, : NHEAD * 64], fgT_d[:, : NHEAD * 64]
            )
            nc.sync.dma_start(
                yb_t[:, NHEAD * BC :], yb_d[:, NHEAD * BC :]
            )
            nc.sync.dma_start(
                deT_t[:, NHEAD * 128 :], deT_d[:, NHEAD * 128 :]
            )
            nc.sync.dma_start(
                fgT_t[:, NHEAD * 64 :], fgT_d[:, NHEAD * 64 :]
            )
            ones_t = cpool.tile([2 * LBLK * E, 1], f32)
            nc.sync.dma_start(ones_t[:], ones_d[:])
            c_t = cpool.tile([1, 1], f32)
            nc.sync.dma_start(c_t[:], c_d[:])
            # all filtered means accumulate here; written out in one DMA
            msb_t = cpool.tile([LBLK * D, NBLK * BC], f32)

            # Both matmul stacks stay exact fp32. Only the final ones-
            # reduction of the squared z runs in FP32r (1 cyc/col), made
            # f32-exact by also accumulating the rounding residual: the
            # [128, BC] rhs stacks rounded squares (rows 0..63) on top of
            # the residuals (rows 64..127), reduced by one ones-vector
            # matmul. FP32r operands must come from rounding producers.
            onesr_t = cpool.tile([2 * LBLK * E, 1], f32r)
            nc.vector.tensor_copy(onesr_t[:], ones_t[:])

            ll_ps = llpool.tile([1, BC], f32)

            # Broadcast-write the shared covariance trajectory to all batch
            # rows: 8 DMAs x 32 rows x 128KB (the memory-bound bulk).
            NGRP = 8
            GRP = BC // NGRP
            for g in range(NGRP):
                out_ap = covs_d[g * GRP : (g + 1) * GRP, :].rearrange(
                    "b (p f) -> p b f", p=128
                )
                nc.sync.dma_start(
                    out_ap, pf_t[:, None, :].broadcast_to([128, GRP, PFREE])
                )

            pending_zsq = None  # defer ll matmul one block for PE slack
            for j in range(NBLK):
                m_ps = ppool.tile([LBLK * D, BC], f32, tag="mps")
                z_ps = ppool.tile([LBLK * E, BC], f32, tag="zps")
                nc.tensor.matmul(
                    m_ps[:],
                    deT_t[:, j * 128 : (j + 1) * 128],
                    yb_t[:, j * BC : (j + 1) * BC],
                    start=True,
                    stop=True,
                )
                nc.tensor.matmul(
                    z_ps[:],
                    fgT_t[:, j * 64 : (j + 1) * 64],
                    yb_t[:, j * BC : (j + 1) * BC],
                    start=True,
                    stop=True,
                )
                if pending_zsq is not None:
                    zr, rr = pending_zsq
                    nc.tensor.matmul(
                        ll_ps[:],
                        onesr_t[: LBLK * E],
                        zr[:],
                        start=(j == 1),
                        stop=False,
                    )
                    nc.tensor.matmul(
                        ll_ps[:],
                        onesr_t[: LBLK * E],
                        rr[:],
                        start=False,
                        stop=False,
                    )
                if j + 1 < NBLK:
                    # next block's carry rows (the sequential chain)
                    nc.vector.tensor_copy(
                        yb_t[:D, (j + 1) * BC : (j + 2) * BC], m_ps[:D, :]
                    )
                nc.vector.tensor_copy(msb_t[:, j * BC : (j + 1) * BC], m_ps[:])
                # f32-exact ll despite the f32r ones-reduction: accumulate
                # the rounded squares plus the exact rounding residual
                zsq = wpool.tile([LBLK * E, BC], f32, tag="zsq")
                nc.scalar.square(zsq[:], z_ps[:])
                zsqr = wpool.tile([LBLK * E, BC], f32r, tag="zsqr")
                nc.vector.tensor_copy(zsqr[:], zsq[:])
                resid = wpool.tile([LBLK * E, BC], f32r, tag="resid")
                nc.vector.tensor_sub(resid[:], zsq[:], zsqr[:].bitcast(f32))
                pending_zsq = (zsqr, resid)
                if j == NBLK // 2:
                    # first half of the means is final — write it out now so
                    # less lands in the kernel-tail receipt window
                    nc.scalar.dma_start(
                        mst_d[:, : (NBLK // 2) * BC],
                        msb_t[:, : (NBLK // 2) * BC],
                    )

            zr, rr = pending_zsq
            nc.tensor.matmul(ll_ps[:], onesr_t[:LBLK * E], zr[:], start=False, stop=False)
            nc.tensor.matmul(ll_ps[:], onesr_t[:LBLK * E], rr[:], start=False, stop=True)
            ll_sb = wpool.tile([1, BC], f32, tag="llsb")
            # ll = -acc - C
            nc.vector.tensor_scalar(
                ll_sb[:],
                ll_ps[:],
                -1.0,
                c_t[0:1, :],
                mybir.AluOpType.mult,
                mybir.AluOpType.subtract,
            )
            nc.scalar.dma_start(ll_d[:], ll_sb[:])
            nc.scalar.dma_start(
                mst_d[:, (NBLK // 2) * BC :], msb_t[:, (NBLK // 2) * BC :]
            )

    nc.compile()
    return nc


def _get_nc():
    if "nc" not in _CACHE:
        _CACHE["nc"] = _build_nc()
    return _CACHE["nc"]


# ----------------------------------------------------------------------------
# Host wrapper
# ----------------------------------------------------------------------------

def _prepare_shared_inputs(P0_0, A, Q, H, R):
    shared = _shared_recursion(P0_0, A, Q, H, R)
    DD, EE, FF, GGm = _block_operators(shared)
    f = np.float32
    dT = DD.transpose(2, 0, 1).reshape(D, NBLK * 128)
    eT = EE.transpose(2, 0, 1).reshape(64, NBLK * 128)
    fT = FF.transpose(2, 0, 1).reshape(D, NBLK * 64)
    gT = GGm.transpose(2, 0, 1).reshape(64, NBLK * 64)
    deT = np.ascontiguousarray(np.concatenate([dT, eT], axis=0), f)
    fgT = np.ascontiguousarray(np.concatenate([fT, gT], axis=0), f)
    pf = np.ascontiguousarray(shared["Pf"].reshape(-1).reshape(128, -1), f)
    ones = np.ones((128, 1), f)
    cconst = np.array([[np.sum(shared["c"])]], f)
    pf_full = shared["Pf"].astype(f)  # [T, D, D]
    return dict(deT=deT, fgT=fgT, pf=pf, ones=ones, cconst=cconst), pf_full


def _numpy_fallback(emissions, m0, P0, A, Q, H, R):
    """General per-batch filter (only used if the fast-path preconditions
    fail, e.g. non-uniform P0 or unexpected shapes)."""
    Bn, T_, E_ = emissions.shape
    D_ = m0.shape[1]
    A64, Q64, H64, R64 = (x.astype(np.float64) for x in (A, Q, H, R))
    m = m0.astype(np.float64)
    P = P0.astype(np.float64)
    lls = np.zeros(Bn)
    means = np.empty((Bn, T_, D_))
    covs = np.empty((Bn, T_, D_, D_))
    for t in range(T_):
        y = emissions[:, t, :].astype(np.float64)
        mp = m @ A64.T
        Pp = np.einsum("ij,bjk,lk->bil", A64, P, A64) + Q64
        mu = mp @ H64.T
        S = np.einsum("ij,bjk,lk->bil", H64, Pp, H64) + R64
        r = y - mu
        L = np.linalg.cholesky(S)
        z = np.linalg.solve(L, r[..., None])[..., 0]
        lls += (
            -0.5 * np.sum(z * z, axis=-1)
            - np.sum(np.log(np.diagonal(L, axis1=-2, axis2=-1)), axis=-1)
            - 0.5 * E_ * LN2PI
        )
        HP = np.einsum("ij,bjk->bik", H64, Pp)
        Kt = np.swapaxes(np.linalg.solve(S, HP), -1, -2)
        m = mp + np.einsum("bij,bj->bi", Kt, r)
        P = Pp - np.einsum("bij,bjk,blk->bil", Kt, S, Kt)
        means[:, t] = m
        covs[:, t] = P
    return (
        lls.astype(np.float32),
        means.astype(np.float32),
        covs.astype(np.float32),
    )


def kernel(emissions, m0, P0, A, Q, H, R):
    emissions = np.asarray(emissions, np.float32)
    m0 = np.asarray(m0, np.float32)
    P0 = np.asarray(P0, np.float32)
    A = np.asarray(A, np.float32)
    Q = np.asarray(Q, np.float32)
    H = np.asarray(H, np.float32)
    R = np.asarray(R, np.float32)

    if (
        emissions.shape != (B, T, E)
        or m0.shape != (B, D)
        or P0.shape != (B, D, D)
        or not (P0 == P0[0]).all()
    ):
        return _numpy_fallback(emissions, m0, P0, A, Q, H, R)

    from concourse.bass_utils import run_bass_kernel_spmd

    shared_ins, _pf_full = _prepare_shared_inputs(P0[0], A, Q, H, R)

    in_maps = make_in_maps(emissions, m0, shared_ins)
    nc = _get_nc()
    res = run_bass_kernel_spmd(nc, in_maps, core_ids=list(range(NCORES))).results
    return gather(res)


def make_in_maps(emissions, m0, shared_ins):
    in_maps = []
    for c in range(NCORES):
        sl = slice(c * BC, (c + 1) * BC)
        em = np.asarray(emissions[sl], np.float32)  # [BC, T, E]
        ybig = np.zeros((D + LBLK * E, NBLK * BC), np.float32)
        ybig[D:] = (
            em.reshape(BC, NBLK, LBLK, E).transpose(2, 3, 1, 0).reshape(64, NBLK * BC)
        )
        ybig[:D, :BC] = np.asarray(m0[sl], np.float32).T  # block-0 carry
        in_maps.append({"ybig": ybig, **shared_ins})
    return in_maps


# inverse of the row-block permutation applied in _block_operators
_PERM_INV = list(range(1, LBLK)) + [0]


def gather(res):
    ll = np.empty((B,), np.float32)
    means = np.empty((B, T, D), np.float32)
    covs = np.empty((B, T, D, D), np.float32)
    for c in range(NCORES):
        sl = slice(c * BC, (c + 1) * BC)
        ll[sl] = res[c]["ll"][0]
        # mstage is [LBLK*D, NBLK*BC] with row-blocks in carry-permuted order
        means[sl] = (
            res[c]["mstage"]
            .reshape(LBLK, D, NBLK, BC)[_PERM_INV]
            .transpose(3, 2, 0, 1)
            .reshape(BC, T, D)
        )
        covs[sl] = res[c]["covs"].reshape(BC, T, D, D)
    return ll, means, covs
